# revision 1
# baseline (speedup 1.0000x reference)
"""Trainium2 Bass kernel for nn_DensityGenerator (B=32, NH=16, H=W=256, 3x3 box conv).

Sharding: pure data-parallel over the batch dim - 4 samples per NeuronCore,
8 cores. Accepts FULL inputs, returns the FULL (32, 256, 256) float32 output.

Per-core structure (S = sample in SBUF as [128 part=(h*8+c), 8192 free=n],
pixel = c*8192 + n):
  Loop 1 (per sample): split-half DMA load; two-level DVE reduce_max with the
    cross-partition max via PE transpose; mx broadcast; is_equal/iota
    accumulations of the winner (chunk, partition) into a shared staging
    tile; per-pixel head sums via 16 accumulating delta-matmuls (pixel-major
    PSUM [128,512]: partition = pixel>>9, free = pixel&511); mean via ScalarE
    accumulate during the PSUM->SBUF copy; separable 3x3 zero-padded box conv
    (free-dim shifts + PE shift-matmuls for cross-partition rows) -> xt.
  Batched winner blocks (one per sample PAIR, so the first two tails
  overlap the matmul stream): all-ones matmul collapses
    the per-partition accumulations; a [4]-wide coordinate chain; ONE 128-row
    indirect-DMA gather of the 4 winner chunks x 16 heads; per-block eq
    scans; block-select matmuls broadcast per-sample correction constants.
  Loop 2 (per sample): outlier correction (maps==mx -> mean) applied
    post-conv as a 9-point box stencil at the winner pixel (conv is linear);
    temperature softmax via ScalarE exp with folded scale 1/(16*9*2) and
    accumulated row sums; reciprocal; ScalarE scale; store.
"""


from contextlib import ExitStack

import numpy as np

import concourse.bass as bass
import concourse.tile as tile
from concourse import mybir

F32 = mybir.dt.float32
I32 = mybir.dt.int32
U32 = mybir.dt.uint32
OP = mybir.AluOpType
AF = mybir.ActivationFunctionType
AX = mybir.AxisListType

B_PER_CORE = 4
NH = 16
HW = 65536
NPB = 8192
SCALE = 1.0 / (16.0 * 9.0 * 2.0)

# const blob column layout: [128, NCONST]
_LEXT = 0        # 143
_IDENT = 143     # 128
_IOTA = 271      # 512
_PVEC = 783      # 1
_SH1 = 784       # 128
_SH2 = 912       # 128
_ONES = 1200     # 128; all-ones on rows 0,32,64,96 (and row 0 for generic use)
_CROW = 1328     # 512
_CCOL = 1840     # 512
_ONESC = 2356    # 1
_BSELT = 2357    # 4:  [128,4]  BSELT[p, m] = (m == p>>5)
_HPAT = 2361     # 1:  (p&31)<16 ? (p&31)*512 : 0
_BOFF = 2362     # 4 on row 0: [0, 8192, 16384, 24576]
_DROW4 = 2366    # 4 on rows 0..3: [-1, 1, -1, 1]
_BSEL4 = 2370    # 128: [4,128] BSEL4[b, p] = (b == p>>5)
_BP2 = 2498      # 64:  [2,64]  BP2[m, q] = (q>>5 == m)  (pair-local block sel)
_B2H = 2562      # 128: [2,128] B2H[m, p] = (p>>5 == m+2) (upper-half block sel)
NCONST = 2690


def make_consts():
    p = np.arange(128)
    c = p & 7
    cb = np.zeros((128, NCONST), np.float32)
    cb[p, _LEXT + c * 16 + 15] = 1.0
    cb[:, _IDENT : _IDENT + 128] = np.eye(128, dtype=np.float32)
    cb[:, _IOTA : _IOTA + 512] = np.arange(512, dtype=np.float32)
    cb[:, _PVEC] = p
    cb[p[:-1], _SH1 + p[:-1] + 1] = 1.0
    cb[p[1:], _SH2 + p[1:] - 1] = 1.0
    cb[np.array([0, 32, 64, 96]), _ONES : _ONES + 128] = 1.0
    f = np.arange(512)
    cb[:, _CROW : _CROW + 512] = 2 * p[:, None] + (f[None, :] >= 256)
    cb[:, _CCOL : _CCOL + 512] = f[None, :] & 255
    cb[:, _ONESC] = 1.0
    cb[:, _BSELT : _BSELT + 4] = (np.arange(4)[None, :] == (p >> 5)[:, None])
    r = p & 31
    cb[:, _HPAT] = np.where(r < 16, r * 512, 0)
    cb[0, _BOFF : _BOFF + 4] = [0.0, 8192.0, 16384.0, 24576.0]
    cb[0:4, _DROW4 : _DROW4 + 4] = np.array([-1.0, 1.0, -1.0, 1.0])[None, :]
    cb[0:4, _BSEL4 : _BSEL4 + 128] = (np.arange(4)[:, None] == (p >> 5)[None, :])
    q = np.arange(64)
    cb[0:2, _BP2 : _BP2 + 64] = (np.arange(2)[:, None] == (q >> 5)[None, :])
    cb[0:2, _B2H : _B2H + 128] = (np.arange(2)[:, None] == ((p >> 5) - 2)[None, :])
    return {"cb": cb}


def build(nc):
    maps = nc.dram_tensor(
        "maps", [B_PER_CORE, NH, 256, 256], F32, kind="ExternalInput"
    ).ap()
    out = nc.dram_tensor(
        "out", [B_PER_CORE, 256, 256], F32, kind="ExternalOutput"
    ).ap()
    cbd = nc.dram_tensor("cb", [128, NCONST], F32, kind="ExternalInput").ap()
    with tile.TileContext(nc) as tc:
        with ExitStack() as ctx:
            _body(ctx, tc, maps, out, cbd)
    return nc


def _body(ctx, tc, maps, out, cbd):
    nc = tc.nc
    pC = ctx.enter_context(tc.tile_pool(name="pC", bufs=1))
    pS = ctx.enter_context(tc.tile_pool(name="pS", bufs=3))
    pM = ctx.enter_context(tc.tile_pool(name="pM", bufs=2))
    pW = ctx.enter_context(tc.tile_pool(name="pW", bufs=3))
    pP = ctx.enter_context(tc.tile_pool(name="pP", bufs=2, space="PSUM"))
    pB = ctx.enter_context(tc.tile_pool(name="pB", bufs=2, space="PSUM"))
    pPm = ctx.enter_context(tc.tile_pool(name="pPm", bufs=2, space="PSUM"))

    cb = pC.tile([128, NCONST], F32, name="cb_sb")
    nc.sync.dma_start(out=cb[:, 0:271], in_=cbd[:, 0:271])
    lext = cb[:, _LEXT : _LEXT + 143]
    ident = cb[:, _IDENT : _IDENT + 128]
    iota = cb[:, _IOTA : _IOTA + 512]
    sh1 = cb[:, _SH1 : _SH1 + 128]
    sh2 = cb[:, _SH2 : _SH2 + 128]
    ones = cb[0:1, _ONES : _ONES + 128]
    crow = cb[:, _CROW : _CROW + 512]
    ccol = cb[:, _CCOL : _CCOL + 512]
    onescol = cb[:, _ONESC : _ONESC + 1]
    bselt = cb[:, _BSELT : _BSELT + 4]
    hpat = cb[:, _HPAT : _HPAT + 1]
    boff = cb[0:1, _BOFF : _BOFF + 4]
    drow4 = cb[0:4, _DROW4 : _DROW4 + 4]
    bsel4 = cb[0:4, _BSEL4 : _BSEL4 + 128]
    bp2 = cb[0:2, _BP2 : _BP2 + 64]
    b2h = cb[0:2, _B2H : _B2H + 128]
    pvec = cb[:, _PVEC : _PVEC + 1]

    flat = maps.rearrange("b nh hh w -> b nh (hh w)")
    maps_cn = flat.rearrange("b nh (c n) -> b (nh c) n", c=8)
    maps_rows = flat.rearrange("b nh (r j) -> (b nh r) j", j=128)
    out_v = out.rearrange("b hh w -> b (hh w)").rearrange("b (p f) -> b p f", f=512)

    # persistent cross-sample staging
    X8 = pC.tile([128, 8], F32, name="X8")
    cb_tail_loaded = [False]
    MX4 = pC.tile([1, 4], F32, name="MX4")
    MXB = pC.tile([128, 4], F32, name="MXB")
    RS4 = pC.tile([128, 4], F32, name="RS4")
    Y4 = pC.tile([128, 2], F32, name="Y4")
    PFB = pC.tile([128, 20], F32, name="PFB")
    xts = []

    # per-pair winner extraction: samples (2*pr, 2*pr+1)
    def winner_pair(pr):
        b0 = 2 * pr
        blocksel = bsel4[0:2, :] if pr == 0 else b2h
        with tc.high_priority():
            KFp = pP.tile([1, 4], F32, name="KFp", tag="tp")
            nc.tensor.matmul(
                out=KFp, lhsT=onescol, rhs=X8[:, 4 * pr : 4 * pr + 4],
                start=True, stop=True,
            )
            KF4 = pW.tile([1, 4], F32, name="KF4")
            nc.vector.tensor_copy(out=KF4, in_=KFp)
            KI4 = pW.tile([1, 4], I32, name="KI4")
            nc.vector.tensor_copy(out=KI4, in_=KF4)
            ci4 = pW.tile([1, 2], I32, name="ci4")
            nc.vector.tensor_scalar(
                out=ci4, in0=KI4[:, 1:4:2], scalar1=7, scalar2=None,
                op0=OP.bitwise_and,
            )
            ci4f = pW.tile([1, 2], F32, name="ci4f")
            nc.vector.tensor_copy(out=ci4f, in_=ci4)
            ra = pW.tile([1, 2], F32, name="ra")  # chat*64 + khat
            nc.vector.scalar_tensor_tensor(
                out=ra, in0=ci4f, scalar=64.0, in1=KF4[:, 0:4:2],
                op0=OP.mult, op1=OP.add,
            )
            ra2 = pW.tile([1, 2], F32, name="ra2")  # + b*8192
            nc.vector.tensor_tensor(
                out=ra2, in0=ra, in1=boff[:, b0 : b0 + 2], op=OP.add
            )
            RAp = pP.tile([2, 1], F32, name="RAp", tag="tp")
            nc.tensor.transpose(out=RAp, in_=ra2, identity=ident[0:1, 0:1])
            RAs = pW.tile([2, 1], F32, name="RAs")
            nc.vector.tensor_copy(out=RAs, in_=RAp)
            RC = pB.tile([128, 4], F32, name="RC", tag="bc")
            nc.tensor.matmul(out=RC[0:64, 0:1], lhsT=bp2, rhs=RAs, start=True, stop=True)
            offs_f4 = pW.tile([64, 1], F32, name="offs_f4")
            nc.vector.tensor_tensor(
                out=offs_f4, in0=hpat[0:64, :], in1=RC[0:64, 0:1], op=OP.add
            )
            offs_i4 = pW.tile([64, 1], U32, name="offs_i4")
            nc.vector.tensor_copy(out=offs_i4, in_=offs_f4)
            G4 = pW.tile([64, 128], F32, name="G4")
            nc.gpsimd.indirect_dma_start(
                out=G4, out_offset=None, in_=maps_rows,
                in_offset=bass.IndirectOffsetOnAxis(ap=offs_i4, axis=0),
                bounds_check=32767, oob_is_err=False,
            )
            Y4 = pW.tile([64, 2], F32, name="Y4")
            nc.vector.memset(Y4, 0.0)
            for i in range(2):
                b = b0 + i
                p0 = 32 * i
                junkB = pW.tile([16, 128], F32, name="junkB")
                nc.vector.scalar_tensor_tensor(
                    out=junkB, in0=G4[p0 : p0 + 16, :],
                    scalar=MXB[p0 : p0 + 16, b : b + 1],
                    in1=iota[p0 : p0 + 16, 0:128],
                    op0=OP.is_equal, op1=OP.mult,
                    accum_out=Y4[p0 : p0 + 16, 0:1],
                )
                junkC = pW.tile([16, 128], F32, name="junkC")
                nc.vector.tensor_scalar(
                    out=junkC, in0=G4[p0 : p0 + 16, :],
                    scalar1=MXB[p0 : p0 + 16, b : b + 1], scalar2=None,
                    op0=OP.is_equal, op1=OP.add,
                    accum_out=Y4[p0 : p0 + 16, 1:2],
                )
            YSp = pP.tile([2, 2], F32, name="YSp", tag="tp")
            nc.tensor.matmul(
                out=YSp, lhsT=bselt[0:64, 0:2], rhs=Y4, start=True, stop=True
            )
            YS = pW.tile([2, 2], F32, name="YS")
            nc.vector.tensor_copy(out=YS, in_=YSp)

            # coords on [2,1] tiles (partitions 0..1)
            kTp = pP.tile([2, 1], F32, name="kTp", tag="tp")
            nc.tensor.transpose(out=kTp, in_=KF4[:, 0:4:2], identity=ident[0:1, 0:1])
            cTp = pP.tile([2, 1], F32, name="cTp", tag="tp")
            nc.tensor.transpose(out=cTp, in_=ci4f, identity=ident[0:1, 0:1])
            kTi = pW.tile([2, 1], I32, name="kTi")
            nc.vector.tensor_copy(out=kTi, in_=kTp)
            cTi = pW.tile([2, 1], I32, name="cTi")
            nc.vector.tensor_copy(out=cTi, in_=cTp)
            ji4 = pW.tile([2, 1], I32, name="ji4")
            nc.vector.tensor_copy(out=ji4, in_=YS[:, 0:1])
            n4 = pW.tile([2, 1], I32, name="n4")
            nc.vector.scalar_tensor_tensor(
                out=n4, in0=kTi, scalar=128, in1=ji4, op0=OP.mult, op1=OP.add
            )
            q84 = pW.tile([2, 1], I32, name="q84")
            nc.vector.tensor_scalar(
                out=q84, in0=n4, scalar1=8, scalar2=None, op0=OP.logical_shift_right
            )
            j4 = pW.tile([2, 1], I32, name="j4")
            nc.vector.tensor_scalar(
                out=j4, in0=n4, scalar1=255, scalar2=None, op0=OP.bitwise_and
            )
            i4 = pW.tile([2, 1], I32, name="i4")
            nc.vector.scalar_tensor_tensor(
                out=i4, in0=cTi, scalar=32, in1=q84, op0=OP.mult, op1=OP.add
            )
            i4f = pW.tile([2, 1], F32, name="i4f")
            nc.vector.tensor_copy(out=i4f, in_=i4)
            j4f = pW.tile([2, 1], F32, name="j4f")
            nc.vector.tensor_copy(out=j4f, in_=j4)
            PFx4 = pW.tile([2, 4], F32, name="PFx4")
            nc.vector.tensor_copy(out=PFx4[:, 0:2], in_=i4f.to_broadcast([2, 2]))
            nc.vector.tensor_copy(out=PFx4[:, 2:4], in_=j4f.to_broadcast([2, 2]))
            PF4 = pW.tile([2, 8], F32, name="PF4")
            nc.vector.tensor_tensor(
                out=PF4[:, 0:4], in0=PFx4, in1=drow4[0:2, :], op=OP.add
            )

            # corr value per sample: (mean - mx) * cnt
            TOTp = pP.tile([1, 2], F32, name="TOTp", tag="tp")
            nc.tensor.matmul(
                out=TOTp, lhsT=onescol, rhs=RS4[:, b0 : b0 + 2],
                start=True, stop=True,
            )
            tt4 = pW.tile([1, 2], F32, name="tt4")
            nc.vector.tensor_scalar(
                out=tt4, in0=TOTp, scalar1=1.0 / (NH * HW), scalar2=None, op0=OP.mult
            )
            c1r = pW.tile([1, 2], F32, name="c1r")
            nc.vector.tensor_tensor(
                out=c1r, in0=tt4, in1=MX4[:, b0 : b0 + 2], op=OP.subtract
            )
            c1Tp = pP.tile([2, 1], F32, name="c1Tp", tag="tp")
            nc.tensor.transpose(out=c1Tp, in_=c1r, identity=ident[0:1, 0:1])
            c1T = pW.tile([2, 1], F32, name="c1T")
            nc.vector.tensor_copy(out=c1T, in_=c1Tp)
            nc.vector.tensor_tensor(
                out=PF4[:, 4:5], in0=c1T, in1=YS[:, 1:2], op=OP.mult
            )

            # two-hop broadcast: [2,5] -> block-local [128,5] -> per-sample
            PF32ps = pB.tile([128, 8], F32, name="PF32ps", tag="bc")
            nc.tensor.matmul(
                out=PF32ps[:, 0:5], lhsT=blocksel, rhs=PF4[:, 0:5],
                start=True, stop=True,
            )
            PF32 = pW.tile([128, 8], F32, name="PF32")
            nc.scalar.copy(out=PF32[:, 0:5], in_=PF32ps[:, 0:5])
            for i in range(2):
                b = b0 + i
                p0 = 32 * b
                PFbps = pB.tile([128, 8], F32, name="PFbps", tag="bc")
                nc.tensor.matmul(
                    out=PFbps[:, 0:5],
                    lhsT=cb[p0 : p0 + 1, _ONES : _ONES + 128],
                    rhs=PF32[p0 : p0 + 1, 0:5],
                    start=True, stop=True, tile_position=(p0, 0),
                )
                nc.scalar.copy(out=PFB[:, 5 * b : 5 * b + 5], in_=PFbps[:, 0:5])

    def loop1_body(b):
        S = pS.tile([128, NPB], F32, name="S")
        nparts = 8
        step = NPB // nparts
        for q in range(nparts):
            nc.sync.dma_start(
                out=S[:, q * step : (q + 1) * step],
                in_=maps_cn[b, :, q * step : (q + 1) * step],
            )
            if not cb_tail_loaded[0]:
                # bulk constants ride behind the first quarter-sample load
                cb_tail_loaded[0] = True
                nc.sync.dma_start(out=cb[:, 271:NCONST], in_=cbd[:, 271:NCONST])

        M1 = pW.tile([128, 64], F32, name="M1")
        kstep = 64 // nparts
        for q in range(nparts):
            nc.vector.tensor_reduce(
                out=M1[:, q * kstep : (q + 1) * kstep],
                in_=S[:, q * step : (q + 1) * step].rearrange(
                    "p (k j) -> p k j", j=128
                ),
                axis=AX.X, op=OP.max,
            )
        pm = pW.tile([128, 1], F32, name="pm")
        nc.vector.tensor_reduce(out=pm, in_=M1, axis=AX.X, op=OP.max)
        with tc.high_priority():
            T1 = pP.tile([1, 128], F32, name="T1", tag="tp")
            nc.tensor.transpose(out=T1, in_=pm, identity=ident)
            nc.vector.tensor_reduce(
                out=MX4[:, b : b + 1], in_=T1, axis=AX.X, op=OP.max
            )
            mxb_ps = pB.tile([128, 4], F32, name="mxb_ps", tag="bc")
            nc.tensor.matmul(
                out=mxb_ps[:, 0:1], lhsT=ones, rhs=MX4[:, b : b + 1],
                start=True, stop=True,
            )
            nc.scalar.copy(out=MXB[:, b : b + 1], in_=mxb_ps[:, 0:1])

            # eq/iota accumulations (collapsed later by one all-ones matmul)
            junkA = pW.tile([128, 64], F32, name="junkA")
            nc.vector.scalar_tensor_tensor(
                out=junkA, in0=M1, scalar=MXB[:, b : b + 1], in1=iota[:, 0:64],
                op0=OP.is_equal, op1=OP.mult, accum_out=X8[:, 2 * b : 2 * b + 1],
            )
            o_col = pW.tile([128, 1], F32, name="o_col")
            nc.vector.tensor_scalar(
                out=o_col, in0=pm, scalar1=MXB[:, b : b + 1], scalar2=None,
                op0=OP.is_equal,
            )
            nc.vector.tensor_tensor(
                out=X8[:, 2 * b + 1 : 2 * b + 2], in0=o_col, in1=pvec, op=OP.mult
            )

        # ---- head sums ----
        psum_m = pPm.tile([128, 512], F32, name="psum_m", tag="pm")
        for t in range(16):
            nc.tensor.matmul(
                out=psum_m,
                lhsT=lext[:, 15 - t : 143 - t],
                rhs=S[:, 512 * t : 512 * (t + 1)],
                start=(t == 0), stop=(t == 15),
            )
        m = pM.tile([128, 512], F32, name="m")
        nc.scalar.activation(
            out=m, in_=psum_m, func=AF.Copy, accum_out=RS4[:, b : b + 1]
        )

        # ---- conv ----
        tH = pM.tile([128, 512], F32, name="tH")
        nc.vector.tensor_tensor(
            out=tH[:, 0:511], in0=m[:, 0:511], in1=m[:, 1:512], op=OP.add
        )
        h = pM.tile([128, 512], F32, name="h")
        nc.vector.tensor_tensor(
            out=h[:, 1:511], in0=tH[:, 0:510], in1=m[:, 2:512], op=OP.add
        )
        nc.scalar.copy(out=h[:, 0:1], in_=tH[:, 0:1])
        nc.scalar.copy(out=h[:, 511:512], in_=tH[:, 510:511])
        nc.scalar.copy(out=h[:, 255:257], in_=tH[:, 254:257:2])
        hsh_ps = pPm.tile([128, 512], F32, name="hsh_ps", tag="hsh")
        nc.tensor.matmul(
            out=hsh_ps[:, 0:256], lhsT=sh1, rhs=h[:, 256:512], start=True, stop=True
        )
        nc.tensor.matmul(
            out=hsh_ps[:, 256:512], lhsT=sh2, rhs=h[:, 0:256], start=True, stop=True
        )
        v1 = pM.tile([128, 256], F32, name="v1")
        nc.vector.tensor_tensor(out=v1, in0=h[:, 0:256], in1=h[:, 256:512], op=OP.add)
        xt = pM.tile([128, 512], F32, name="xt", tag="xt", bufs=4)
        nc.vector.tensor_tensor(out=xt[:, 0:256], in0=v1, in1=hsh_ps[:, 0:256], op=OP.add)
        nc.vector.tensor_tensor(
            out=xt[:, 256:512], in0=v1, in1=hsh_ps[:, 256:512], op=OP.add
        )
        xts.append(xt)

    def loop2_body(b):
        xt = xts[b]
        PFb = PFB[:, 5 * b : 5 * b + 5]
        rlA = pW.tile([128, 1], F32, name="rlA", bufs=4)
        nc.vector.tensor_scalar(
            out=rlA, in0=crow[:, 0:1], scalar1=PFb[:, 0:1], scalar2=None, op0=OP.is_ge
        )
        wl = pW.tile([128, 1], F32, name="wl", bufs=4)
        nc.vector.scalar_tensor_tensor(
            out=wl, in0=crow[:, 0:1], scalar=PFb[:, 1:2], in1=rlA,
            op0=OP.is_le, op1=OP.mult,
        )
        rhA = pW.tile([128, 1], F32, name="rhA", bufs=4)
        nc.vector.tensor_scalar(
            out=rhA, in0=crow[:, 256:257], scalar1=PFb[:, 0:1], scalar2=None,
            op0=OP.is_ge,
        )
        wh = pW.tile([128, 1], F32, name="wh", bufs=4)
        nc.vector.scalar_tensor_tensor(
            out=wh, in0=crow[:, 256:257], scalar=PFb[:, 1:2], in1=rhA,
            op0=OP.is_le, op1=OP.mult,
        )
        nc.vector.tensor_tensor(out=wl, in0=wl, in1=PFb[:, 4:5], op=OP.mult)
        nc.vector.tensor_tensor(out=wh, in0=wh, in1=PFb[:, 4:5], op=OP.mult)
        colA = pM.tile([128, 256], F32, name="colA", bufs=4)
        nc.vector.tensor_scalar(
            out=colA, in0=ccol[:, 0:256], scalar1=PFb[:, 2:3], scalar2=None,
            op0=OP.is_ge,
        )
        colm = pM.tile([128, 256], F32, name="colm", bufs=4)
        nc.vector.scalar_tensor_tensor(
            out=colm, in0=ccol[:, 0:256], scalar=PFb[:, 3:4], in1=colA,
            op0=OP.is_le, op1=OP.mult,
        )
        x2 = pM.tile([128, 512], F32, name="x2", bufs=4)
        nc.vector.scalar_tensor_tensor(
            out=x2[:, 0:256], in0=colm, scalar=wl, in1=xt[:, 0:256],
            op0=OP.mult, op1=OP.add,
        )
        nc.vector.scalar_tensor_tensor(
            out=x2[:, 256:512], in0=colm, scalar=wh, in1=xt[:, 256:512],
            op0=OP.mult, op1=OP.add,
        )
        e = pM.tile([128, 512], F32, name="e", bufs=4)
        erow = pW.tile([128, 2], F32, name="erow", bufs=4)
        nc.scalar.activation(
            out=e[:, 0:256], in_=x2[:, 0:256], func=AF.Exp, scale=SCALE,
            accum_out=erow[:, 0:1],
        )
        nc.scalar.activation(
            out=e[:, 256:512], in_=x2[:, 256:512], func=AF.Exp, scale=SCALE,
            accum_out=erow[:, 1:2],
        )
        T8 = pP.tile([1, 2], F32, name="T8", tag="tp")
        nc.tensor.matmul(out=T8, lhsT=onescol, rhs=erow, start=True, stop=True)
        se = pW.tile([1, 1], F32, name="se", bufs=4)
        nc.vector.tensor_reduce(out=se, in_=T8, axis=AX.X, op=OP.add)
        rec = pW.tile([1, 1], F32, name="rec", bufs=4)
        nc.vector.reciprocal(out=rec, in_=se)
        recb_ps = pB.tile([128, 8], F32, name="recb_ps", tag="bc")
        nc.tensor.matmul(out=recb_ps[:, 0:1], lhsT=ones, rhs=rec, start=True, stop=True)
        recb = pW.tile([128, 1], F32, name="recb", bufs=4)
        nc.scalar.copy(out=recb, in_=recb_ps[:, 0:1])
        outt = pM.tile([128, 512], F32, name="outt", bufs=4)
        nc.vector.tensor_scalar(
            out=outt, in0=e, scalar1=recb, scalar2=None, op0=OP.mult
        )
        nc.sync.dma_start(out=out_v[b], in_=outt)

    loop1_body(0)
    loop1_body(1)
    winner_pair(0)
    loop2_body(0)
    loop2_body(1)
    loop1_body(2)
    loop1_body(3)
    winner_pair(1)
    loop2_body(2)
    loop2_body(3)


# ---- neuronxcc single-sync-wait workaround ----

import bass_rust
from concourse import mybir


def split_multiwaits(nc, limit=1):
    uid = [0]

    def mk_nop(base, engine, on_wait, on_update):
        uid[0] += 1
        return mybir.InstNoOp(
            name=f"{base}_wsplit{uid[0]}",
            engine=engine,
            bass_nofuse=True,
            sync_info=mybir.SyncInfo(on_wait=on_wait, on_update=on_update),
        )

    n_split = 0
    for f in nc.m.functions:
        new_blocks = []
        for blk in f.blocks:
            out = []
            for inst in blk.instructions:
                si = inst.sync_info
                pre, post = [], []
                if si is not None:
                    waits = list(si.on_wait) if si.on_wait else []
                    ups = list(si.on_update) if si.on_update else []
                    if len(waits) > limit or len(ups) > limit:
                        n_split += 1
                        keep_w = waits[-limit:] if waits else []
                        for w in waits[: len(waits) - len(keep_w)]:
                            pre.append(mk_nop(inst.name, inst.engine, [w], []))
                        keep_u = ups[:limit]
                        for u in ups[limit:]:
                            post.append(mk_nop(inst.name, inst.engine, [], [u]))
                        inst.sync_info = mybir.SyncInfo(
                            on_wait=keep_w, on_update=keep_u
                        )
                out.extend(pre)
                out.append(inst)
                out.extend(post)
            new_blocks.append(
                bass_rust.BasicBlock(
                    name=blk.name,
                    instructions=out,
                    IsExit=blk.IsExit,
                    IsLoopEntry=blk.IsLoopEntry,
                    IsPredicated=blk.IsPredicated,
                )
            )
        f.blocks = new_blocks
    return n_split




_CACHED = None


def _get_nc():
    global _CACHED
    if _CACHED is None:
        nc = bass.Bass(
            trn_type="TRN2", target_bir_lowering=False, debug=False, num_devices=1
        )
        build(nc)
        split_multiwaits(nc)
        _CACHED = nc
    return _CACHED


def run(maps, trace=False):
    """maps: full (32, 16, 256, 256) float32. Returns (out, exec_time_ns)."""
    from concourse.bass_utils import run_bass_kernel_spmd

    maps = np.ascontiguousarray(maps, dtype=np.float32)
    n_cores = 8
    assert maps.shape == (n_cores * B_PER_CORE, NH, 256, 256)
    nc = _get_nc()
    consts = make_consts()
    in_maps = [
        {"maps": maps[c * B_PER_CORE : (c + 1) * B_PER_CORE], **consts}
        for c in range(n_cores)
    ]
    res = run_bass_kernel_spmd(nc, in_maps, list(range(n_cores)), trace=trace)
    out = np.concatenate([res.results[c]["out"] for c in range(n_cores)], axis=0)
    return out, res.exec_time_ns


def kernel(maps, conv_weight=None, **_ignored):
    # conv_weight is the fixed ones/9 box filter installed by the module at
    # init (spec fill: ones, normalized by 1/K^2 in setup) - folded into the
    # kernel's constant SCALE.
    out, _ = run(maps)
    return out

